# Initial kernel scaffold
#
"""Self-contained Trainium2 (Bass) kernel for the BaseSigKernel problem.

kernel(xs, ys) -> (24, 24) float32 signature-kernel Gram matrix.

Math (per (x,y) pair; Salvi et al. finite-difference scheme, dyadic_order=1):
    a[r, s]   = <dy[r], dx[s]> / 4          (190x190, dyadic 2x2-duplicated)
    c1 = 1 + a/2 + a^2/12 ;  c2 = 1 - a^2/12
    u[0, :] = u[:, 0] = 1
    u[r+1, s+1] = (u[r+1, s] + u[r, s+1]) * c1[r, s] - u[r, s] * c2[r, s]
    result = u[190, 190]

Distribution: data-parallel over the batch_x axis - core ci owns b in
{3ci, 3ci+1, 3ci+2} x all 24 c's = 72 pairs, held on SBUF partitions
(three 32-partition bands; 24 used per band, the rest compute on zero
padding).

Per core, rows are processed serially; each row is ONE interleaved DVE
tensor_tensor_scan of length 380 alternating
    step 2s  : state = 1     * state + u_prev[s+1]
    step 2s+1: state = c1[s] * state + (-c2[s] * u_prev[s])
which reproduces the reference f32 association (u_left+u_up)*c1 - u_diag*c2
exactly. The scan's data1 is ubuf_prev[3:383] itself: u rows are stored
stride-2 (u[k] at ubuf[2k+1]) and one DVE multiply writes -c2*u into the
dead even lanes. Coefficient rows are produced just-in-time from tiny K=8
TensorE matmuls through ScalarE + GpSimd; one coefficient row serves two
PDE rows (dyadic duplication).
"""

import math
from contextlib import ExitStack

import numpy as np

import concourse.bacc as bacc
import concourse.mybir as mybir
import concourse.tile as tile
from concourse.ap import AP

F32 = mybir.dt.float32
Alu = mybir.AluOpType
Act = mybir.ActivationFunctionType

BX, BY, L, DIM = 24, 24, 96, 8
N_CORES = 8
BB = BX // N_CORES          # 3 b-values per core
BAND = 32                   # matmul output base partitions must be 0/32/64
P = BB * BAND               # 96 partitions; 24..31, 56..63, 88..95 are c-padding
NH = L - 1                  # 95: half-resolution grid length
NF = 2 * NH                 # 190: full-resolution grid length
INV_SQRT12 = 1.0 / math.sqrt(12.0)
CF_B = 384                  # coeff slot: [0:380) = [1|c1] interleaved, [384:574) c2neg
UW = 2 * NF + 4             # u row buffer width (384): u[k] at ubuf[2k+1]


def _view(t_ap: AP, off: int, dims) -> AP:
    """Custom AP view of a tile: dims = [(step, count), ...] incl partition dim."""
    return AP(t_ap.tensor, t_ap.offset + off, [list(d) for d in dims])


def build_bass(ring: int = 4):
    nc = bacc.Bacc()
    # dyT and dxT packed into one tensor -> one DMA -> one PE sync wait
    inp_d = nc.declare_dram_parameter("inp", [DIM, NH * BAND + BB * NH], F32, isOutput=False)
    out_d = nc.declare_dram_parameter("out", [P, 1], F32, isOutput=True)

    with ExitStack() as ctx:
        tc = ctx.enter_context(tile.TileContext(nc))
        sbuf = ctx.enter_context(tc.tile_pool(name="sbuf", bufs=1))
        psum = ctx.enter_context(tc.tile_pool(name="psum", bufs=3, space="PSUM"))

        inp_t = sbuf.tile([DIM, NH * BAND + BB * NH], F32, name="inp_t", tag="inp_t")
        nc.gpsimd.dma_start(inp_t[:], inp_d[:])

        # u rows, stride-2 storage: u[k] = ubuf[2k+1]; scan writes [2:382);
        # position 1 is the left boundary u[0] = 1 (preset, never written).
        ub = [sbuf.tile([P, UW], F32, name=f"u{i}", tag=f"u{i}") for i in range(2)]
        nc.vector.memset(ub[0][:], 1.0)   # row 0 = all ones
        nc.vector.memset(ub[1][:], 1.0)

        cfs = [
            sbuf.tile([P, CF_B + NF], F32, name=f"cf{i}", tag=f"cf{i}")
            for i in range(ring)
        ]
        t2s = [
            sbuf.tile([P, NH], F32, name=f"t2{i}", tag=f"t2{i}") for i in range(ring)
        ]
        c1hs = [
            sbuf.tile([P, NH], F32, name=f"c1h{i}", tag=f"c1h{i}") for i in range(ring)
        ]
        s12s = [
            sbuf.tile([P, NH], F32, name=f"s12{i}", tag=f"s12{i}")
            for i in range(ring)
        ]

        # interleaved scan-coefficient even lanes are the constant 1.0
        for cf in cfs:
            cp_step, _ = cf.ap[0]
            nc.gpsimd.memset(_view(cf, 0, [(cp_step, P), (2, NF)]), 1.0)

        # per-partition bias constant -1.0 for ACT
        cbias = sbuf.tile([P, 1], F32, name="cbias", tag="cbias")
        nc.gpsimd.memset(cbias[:], -1.0)

        def bcast_h(t_ap):
            # [P, NH] -> [P, NH, 2] with the last dim broadcast (step 0)
            p_step, p_cnt = t_ap.ap[0]
            return _view(t_ap, 0, [(p_step, p_cnt), (1, NH), (0, 2)])

        def produce_coeff(q):
            """One half-resolution coefficient row; serves PDE rows 2q, 2q+1."""
            pa_full = psum.tile([P, 512], F32, name="pa", tag="pa")
            pa = pa_full[:, 0:NH]
            lhsT = inp_t[:, q * BAND : (q + 1) * BAND]     # [8, 32] (24 real + 8 pad)
            for b in range(BB):
                nc.tensor.matmul(
                    pa[b * BAND : (b + 1) * BAND, :],
                    lhsT,
                    inp_t[:, NH * BAND + b * NH : NH * BAND + (b + 1) * NH],
                )
            cf, t2, s12 = cfs[q % ring], t2s[q % ring], s12s[q % ring]
            c1h = c1hs[q % ring]
            cp_step, _ = cf.ap[0]
            # s12 = (a * 1/sqrt(12))^2 = a^2/12
            nc.scalar.activation(s12[:], pa[:], Act.Square, scale=INV_SQRT12)
            # c2neg = s12 - 1, expanded 95->190 into cf[384:574)
            cf_c2w = _view(cf, CF_B, [(cp_step, P), (2, NH), (1, 2)])
            nc.scalar.activation(cf_c2w, bcast_h(s12), Act.Identity, bias=cbias[:])
            # t2 = 0.5*a + 1
            nc.scalar.activation(t2[:], pa[:], Act.Identity, bias=1.0, scale=0.5)
            # c1 = t2 + s12 = 1 + a/2 + a^2/12, half-res on GpSimd (short op:
            # the Pool<->DVE shared SBUF port contends with the scans) ...
            nc.gpsimd.tensor_tensor(c1h[:], t2[:], s12[:], Alu.add)
            # ... then expanded into the odd lanes of cf[0:380) by ScalarE
            cf_c1w = _view(cf, 1, [(cp_step, P), (4, NH), (2, 2)])
            nc.scalar.activation(cf_c1w, bcast_h(c1h), Act.Copy)

        def consume_row(r):
            cf = cfs[(r // 2) % ring]
            up = ub[r % 2]
            un = ub[(r + 1) % 2]
            u_step, _ = up.ap[0]
            # write c2neg[s]*u_prev[s] into the DEAD even lanes of ubuf_prev
            # (they hold last row's scan intermediates), so that
            # ubuf_prev[3:383] is exactly the interleaved scan data1:
            #   t=2s   -> ubuf[3+2s] = u_prev[s+1]
            #   t=2s+1 -> ubuf[4+2s] = c2neg[s]*u_prev[s]
            nc.vector.tensor_tensor(
                _view(up, 4, [(u_step, P), (2, NF)]),
                cf[:, CF_B : CF_B + NF],
                _view(up, 1, [(u_step, P), (2, NF)]),
                Alu.mult,
            )
            # interleaved scan: state=(d0*state)+d1 over 380 steps
            nc.vector.tensor_tensor_scan(
                un[:, 2 : 2 + 2 * NF],
                cf[:, 0 : 2 * NF],
                up[:, 3 : 3 + 2 * NF],
                1.0,
                Alu.mult,
                Alu.add,
            )

        # interleave production (lookahead AH slots) with consumption so
        # trace order matches dataflow.
        AH = ring - 2
        for q in range(AH):
            produce_coeff(q)
        for r in range(NF):
            if r % 2 == 0 and r // 2 + AH < NH:
                produce_coeff(r // 2 + AH)
            consume_row(r)

        nc.gpsimd.dma_start(out_d[:], ub[NF % 2][:, 2 * NF + 1 : 2 * NF + 2])

    nc.compile()
    return nc


def pack_inputs(xs: np.ndarray, ys: np.ndarray):
    """Full inputs -> per-core in_maps for run_bass_kernel_spmd."""
    xs = np.asarray(xs, np.float32)
    ys = np.asarray(ys, np.float32)
    dx = np.diff(xs, axis=1) * 0.5            # (24, 95, 8)
    dy = np.diff(ys, axis=1) * 0.5            # (24, 95, 8)
    dyT = np.zeros((DIM, NH, BAND), np.float32)
    dyT[:, :, :BY] = dy.transpose(2, 1, 0)
    dyT = dyT.reshape(DIM, NH * BAND)
    in_maps = []
    for ci in range(N_CORES):
        dxc = dx[ci * BB : (ci + 1) * BB]     # (3, 95, 8)
        dxT = dxc.transpose(2, 0, 1).reshape(DIM, BB * NH)
        inp = np.ascontiguousarray(np.concatenate([dyT, dxT], axis=1))
        in_maps.append({"inp": inp})
    return in_maps


def unpack_outputs(results) -> np.ndarray:
    """Per-core (96,1) outputs -> full (24,24)."""
    out = np.zeros((BX, BY), np.float32)
    for ci in range(N_CORES):
        res = np.asarray(results[ci]["out"]).reshape(P)
        for b in range(BB):
            out[ci * BB + b, :] = res[b * BAND : b * BAND + BY]
    return out


_NC_CACHE = None


def kernel(xs: np.ndarray, ys: np.ndarray) -> np.ndarray:
    """Full (24,96,8) inputs -> full (24,24) output, computed on 8 trn2 cores."""
    global _NC_CACHE
    from concourse.bass_utils import run_bass_kernel_spmd

    if _NC_CACHE is None:
        _NC_CACHE = build_bass()
    in_maps = pack_inputs(xs, ys)
    r = run_bass_kernel_spmd(_NC_CACHE, in_maps, list(range(N_CORES)))
    return unpack_outputs(r.results)



# revision 4
# speedup vs baseline: 1.0378x; 1.0378x over previous
"""Self-contained Trainium2 (Bass) kernel for the BaseSigKernel problem.

kernel(xs, ys) -> (24, 24) float32 signature-kernel Gram matrix.

Math (per (x,y) pair; Salvi et al. finite-difference scheme, dyadic_order=1):
    a[r, s]   = <dy[r], dx[s]> / 4          (190x190, dyadic 2x2-duplicated)
    c1 = 1 + a/2 + a^2/12 ;  c2 = 1 - a^2/12
    u[0, :] = u[:, 0] = 1
    u[r+1, s+1] = (u[r+1, s] + u[r, s+1]) * c1[r, s] - u[r, s] * c2[r, s]
    result = u[190, 190]

Distribution: data-parallel over the batch_x axis - core ci owns b in
{3ci, 3ci+1, 3ci+2} x all 24 c's = 72 pairs, held on SBUF partitions
(three 32-partition bands; 24 used per band, the rest compute on zero
padding).

Per core, rows are processed serially; each row is ONE interleaved DVE
tensor_tensor_scan of length 380 alternating
    step 2s  : state = 1     * state + u_prev[s+1]
    step 2s+1: state = c1[s] * state + (-c2[s] * u_prev[s])
which reproduces the reference f32 association (u_left+u_up)*c1 - u_diag*c2
exactly. The scan's data1 is ubuf_prev[3:383] itself: u rows are stored
stride-2 (u[k] at ubuf[2k+1]) and one DVE multiply writes -c2*u into the
dead even lanes.

Coefficient production (off the DVE critical path):
  - ONE block-diagonal matmul per coefficient row: lhsT [24, 96] holds the
    three bands' dy row-q slices on the block diagonal, rhs [24, 95] stacks
    the three bands' dx, so a single PE instruction fills all 96 output
    partitions (vs 3 per-band matmuls).
  - c1 = Square(a/sqrt(12) + sqrt(3)/2) + 1/4: one ScalarE Square, with the
    +1/4 folded into the dyadic-expand ACT's Identity bias; no cross-tensor
    add needed.
  - c2neg = Square(a/sqrt(12)) - 1: Square + expand-with-bias, as before.
"""

import math
from contextlib import ExitStack

import numpy as np

import concourse.bacc as bacc
import concourse.mybir as mybir
import concourse.tile as tile
from concourse.ap import AP

F32 = mybir.dt.float32
Alu = mybir.AluOpType
Act = mybir.ActivationFunctionType

BX, BY, L, DIM = 24, 24, 96, 8
N_CORES = 8
BB = BX // N_CORES          # 3 b-values per core
BAND = 32                   # matmul output bands of 32 partitions
P = BB * BAND               # 96 partitions; 24..31, 56..63, 88..95 are c-padding
NH = L - 1                  # 95: half-resolution grid length
NF = 2 * NH                 # 190: full-resolution grid length
K = BB * DIM                # 24: stacked contraction dim of the merged matmul
INV_SQRT12 = 1.0 / math.sqrt(12.0)
SQRT3_2 = math.sqrt(3.0) / 2.0
CF_B = 384                  # coeff slot: [0:380) = [1|c1] interleaved, [384:574) c2neg
UW = 2 * NF + 4             # u row buffer width (384): u[k] at ubuf[2k+1]
NQ_HEAD = 6                 # coeff rows whose lhsT arrives in the priority DMA


def _view(t_ap: AP, off: int, dims) -> AP:
    """Custom AP view of a tile: dims = [(step, count), ...] incl partition dim."""
    return AP(t_ap.tensor, t_ap.offset + off, [list(d) for d in dims])


def build_bass(ring: int = 6):
    nc = bacc.Bacc()
    rhs_d = nc.declare_dram_parameter("rhs", [K, NH], F32, isOutput=False)
    lhsA_d = nc.declare_dram_parameter("lhsA", [K, NQ_HEAD * P], F32, isOutput=False)
    lhsB_d = nc.declare_dram_parameter(
        "lhsB", [K, (NH - NQ_HEAD) * P], F32, isOutput=False
    )
    out_d = nc.declare_dram_parameter("out", [P, 1], F32, isOutput=True)

    with ExitStack() as ctx:
        tc = ctx.enter_context(tile.TileContext(nc))
        sbuf = ctx.enter_context(tc.tile_pool(name="sbuf", bufs=1))
        psum = ctx.enter_context(tc.tile_pool(name="psum", bufs=3, space="PSUM"))

        rhs_t = sbuf.tile([K, NH], F32, name="rhs_t", tag="rhs_t")
        lhsA_t = sbuf.tile([K, NQ_HEAD * P], F32, name="lhsA_t", tag="lhsA_t")
        lhsB_t = sbuf.tile(
            [K, (NH - NQ_HEAD) * P], F32, name="lhsB_t", tag="lhsB_t"
        )
        nc.gpsimd.dma_start(rhs_t[:], rhs_d[:])
        nc.gpsimd.dma_start(lhsA_t[:], lhsA_d[:])
        nc.gpsimd.dma_start(lhsB_t[:], lhsB_d[:])

        # u rows, stride-2 storage: u[k] = ubuf[2k+1]; scan writes [2:382);
        # position 1 is the left boundary u[0] = 1 (preset, never written).
        ub = [sbuf.tile([P, UW], F32, name=f"u{i}", tag=f"u{i}") for i in range(2)]
        nc.vector.memset(ub[0][:], 1.0)   # row 0 = all ones
        nc.vector.memset(ub[1][:], 1.0)

        cfs = [
            sbuf.tile([P, CF_B + NF], F32, name=f"cf{i}", tag=f"cf{i}")
            for i in range(ring)
        ]
        chs = [
            sbuf.tile([P, NH], F32, name=f"ch{i}", tag=f"ch{i}") for i in range(ring)
        ]
        s12s = [
            sbuf.tile([P, NH], F32, name=f"s12{i}", tag=f"s12{i}")
            for i in range(ring)
        ]

        # interleaved scan-coefficient even lanes are the constant 1.0
        for cf in cfs:
            cp_step, _ = cf.ap[0]
            nc.gpsimd.memset(_view(cf, 0, [(cp_step, P), (2, NF)]), 1.0)

        # per-partition bias constants for the ACTs
        b_s32 = sbuf.tile([P, 1], F32, name="b_s32", tag="b_s32")
        b_q = sbuf.tile([P, 1], F32, name="b_q", tag="b_q")
        b_n1 = sbuf.tile([P, 1], F32, name="b_n1", tag="b_n1")
        nc.gpsimd.memset(b_s32[:], SQRT3_2)
        nc.gpsimd.memset(b_q[:], 0.25)
        nc.gpsimd.memset(b_n1[:], -1.0)

        def bcast_h(t_ap):
            # [P, NH] -> [P, NH, 2] with the last dim broadcast (step 0)
            p_step, p_cnt = t_ap.ap[0]
            return _view(t_ap, 0, [(p_step, p_cnt), (1, NH), (0, 2)])

        def produce_coeff(q):
            """One half-resolution coefficient row; serves PDE rows 2q, 2q+1."""
            pa_full = psum.tile([P, 512], F32, name="pa", tag="pa")
            pa = pa_full[:, 0:NH]
            if q < NQ_HEAD:
                lhsT = lhsA_t[:, q * P : (q + 1) * P]
            else:
                qq = q - NQ_HEAD
                lhsT = lhsB_t[:, qq * P : (qq + 1) * P]
            nc.tensor.matmul(pa, lhsT, rhs_t[:, 0:NH])
            cf, ch, s12 = cfs[q % ring], chs[q % ring], s12s[q % ring]
            cp_step, _ = cf.ap[0]
            # ch = (a/sqrt12 + sqrt3/2)^2 = a^2/12 + a/2 + 3/4  (= c1 - 1/4)
            nc.scalar.activation(ch[:], pa[:], Act.Square, bias=b_s32[:],
                                 scale=INV_SQRT12)
            # c1 = ch + 1/4, expanded 95->190 into the odd lanes of cf[0:380)
            cf_c1w = _view(cf, 1, [(cp_step, P), (4, NH), (2, 2)])
            nc.scalar.activation(cf_c1w, bcast_h(ch), Act.Identity, bias=b_q[:])
            # s12 = (a/sqrt12)^2 = a^2/12
            nc.scalar.activation(s12[:], pa[:], Act.Square, scale=INV_SQRT12)
            # c2neg = s12 - 1, expanded 95->190 into cf[384:574)
            cf_c2w = _view(cf, CF_B, [(cp_step, P), (2, NH), (1, 2)])
            nc.scalar.activation(cf_c2w, bcast_h(s12), Act.Identity, bias=b_n1[:])

        def consume_row(r):
            cf = cfs[(r // 2) % ring]
            up = ub[r % 2]
            un = ub[(r + 1) % 2]
            u_step, _ = up.ap[0]
            # write c2neg[s]*u_prev[s] into the DEAD even lanes of ubuf_prev
            # (they hold last row's scan intermediates), so that
            # ubuf_prev[3:383] is exactly the interleaved scan data1:
            #   t=2s   -> ubuf[3+2s] = u_prev[s+1]
            #   t=2s+1 -> ubuf[4+2s] = c2neg[s]*u_prev[s]
            nc.vector.tensor_tensor(
                _view(up, 4, [(u_step, P), (2, NF)]),
                cf[:, CF_B : CF_B + NF],
                _view(up, 1, [(u_step, P), (2, NF)]),
                Alu.mult,
            )
            # interleaved scan: state=(d0*state)+d1 over 380 steps
            nc.vector.tensor_tensor_scan(
                un[:, 2 : 2 + 2 * NF],
                cf[:, 0 : 2 * NF],
                up[:, 3 : 3 + 2 * NF],
                1.0,
                Alu.mult,
                Alu.add,
            )

        # interleave production (lookahead AH slots) with consumption so
        # trace order matches dataflow.
        AH = ring - 2
        for q in range(AH):
            produce_coeff(q)
        for r in range(NF):
            if r % 2 == 0 and r // 2 + AH < NH:
                produce_coeff(r // 2 + AH)
            consume_row(r)

        nc.gpsimd.dma_start(out_d[:], ub[NF % 2][:, 2 * NF + 1 : 2 * NF + 2])

    nc.compile()
    return nc


def pack_inputs(xs: np.ndarray, ys: np.ndarray):
    """Full inputs -> per-core in_maps for run_bass_kernel_spmd."""
    xs = np.asarray(xs, np.float32)
    ys = np.asarray(ys, np.float32)
    dx = np.diff(xs, axis=1) * 0.5            # (24, 95, 8)
    dy = np.diff(ys, axis=1) * 0.5            # (24, 95, 8)
    # block-diagonal stationary tensor, shared by all cores:
    # lhs[band*8+d, q*96 + band*32 + c] = dy[c, q, d]
    dyT = dy.transpose(2, 1, 0)               # (8, 95, 24)
    lhs = np.zeros((BB, DIM, NH, BB, BAND), np.float32)
    for band in range(BB):
        lhs[band, :, :, band, :BY] = dyT
    lhs = lhs.reshape(K, NH * P)
    lhsA = np.ascontiguousarray(lhs[:, : NQ_HEAD * P])
    lhsB = np.ascontiguousarray(lhs[:, NQ_HEAD * P :])
    in_maps = []
    for ci in range(N_CORES):
        dxc = dx[ci * BB : (ci + 1) * BB]     # (3, 95, 8)
        rhs = np.ascontiguousarray(
            dxc.transpose(0, 2, 1).reshape(K, NH)
        )
        in_maps.append({"rhs": rhs, "lhsA": lhsA, "lhsB": lhsB})
    return in_maps


def unpack_outputs(results) -> np.ndarray:
    """Per-core (96,1) outputs -> full (24,24)."""
    out = np.zeros((BX, BY), np.float32)
    for ci in range(N_CORES):
        res = np.asarray(results[ci]["out"]).reshape(P)
        for b in range(BB):
            out[ci * BB + b, :] = res[b * BAND : b * BAND + BY]
    return out


_NC_CACHE = None


def kernel(xs: np.ndarray, ys: np.ndarray) -> np.ndarray:
    """Full (24,96,8) inputs -> full (24,24) output, computed on 8 trn2 cores."""
    global _NC_CACHE
    from concourse.bass_utils import run_bass_kernel_spmd

    if _NC_CACHE is None:
        _NC_CACHE = build_bass()
    in_maps = pack_inputs(xs, ys)
    r = run_bass_kernel_spmd(_NC_CACHE, in_maps, list(range(N_CORES)))
    return unpack_outputs(r.results)
